# revision 1
# baseline (speedup 1.0000x reference)
"""Trainium2 Bass kernel for nn_AnchorDeformAtt (deformable anchor attention).

Sharding: spatial L-shard across 8 cores — core i handles image rows
[8i, 8i+8) == pixels l in [512i, 512(i+1)) for BOTH batches and ALL heads.
Zero collectives; the host concatenates per-core output shards.

Per-core pipeline:
  1. PE convs: value memory channel-major [128 = 4heads*32ch, 4096] per
     (b, head-group); offset/size convs (h,p)-major; attn conv pixel-major
     (softmax over the 16 points on the free dim), PE-transposed back.
  2. Grid math on DVE/ACT -> per-tap flat int16 indices + combined weights
     u = attn * bilinear_weight, staged to DRAM as [b, tap, (h,p), l].
  3. idx re-read "wrapped" per 16-partition group for GPSIMD ap_gather
     (stream i = (tap, p, l_hi, q), q = l%16); u re-read broadcast across
     each head's 32 channels with a [[0,32],[1,N]] DMA.
  4. ap_gather per (b, hg): channels=128 = 4 heads x 32ch; each head's index
     stream duplicated on its two 16-partition groups.
  5. DVE: sG = G * u_bcast.
  6. PE: tap-combine + out_proj fused: psum[o, l] += WoutT_chunk.T @ sG_slot,
     accumulated over all (hg, tap, p) slots — contraction over channels.
  7. BN affine inside the ACT PSUM->SBUF copy; DMA out.

Two builds from one builder:
  - use_bf16=False, d_pairs=False: exact f32 build (simulator validation).
  - use_bf16=True, d_pairs=True: perf build — 4-byte gather granularity needs
    d=2, via overlapping-pairs memory (pair j = (mem[j], mem[j+1])), which
    also fetches the (x0, x0+1) taps with a single index.
"""
from contextlib import ExitStack

import numpy as np

import concourse.bass as bass
import concourse.mybir as mybir
import concourse.tile as tile
from concourse import bacc
from concourse.bass_utils import run_bass_kernel_spmd

NH, NP = 8, 16
B, C, H, W = 2, 256, 64, 64
L = H * W            # 4096
NCORES = 8
LSH = L // NCORES    # 512
NLB = LSH // 128     # l-blocks per batch
EPS = 1e-6
F32 = mybir.dt.float32
I16 = mybir.dt.int16

_GRAPH_CACHE = {}


def build_graph(use_bf16=True, d_pairs=True, stub_gather=False, dma_g=False,
                d4=False):
    key = (use_bf16, d_pairs, stub_gather, dma_g, d4)
    if d4:
        assert use_bf16 and d_pairs and not dma_g
    if dma_g:
        assert use_bf16 and d_pairs
    if key in _GRAPH_CACHE:
        return _GRAPH_CACHE[key]
    DT = mybir.dt.bfloat16 if use_bf16 else F32
    DP = 2 if d_pairs else 1          # gather inner d
    NT = 4 // DP                      # tap-groups in the stream
    NIDX_H = NT * NP * LSH            # gather indices per (b, h)
    CHUNK_I = 1024                    # indices per ap_gather instruction

    nc = bacc.Bacc("TRN2", target_bir_lowering=False, debug=False,
                   num_devices=NCORES,
                   dynamic_dma_scratch_size=16384)
    dp = nc.declare_dram_parameter
    feat = dp("feat", [128, B, 2, L], F32, isOutput=False)
    feat_sh = dp("feat_sh", [128, B, 2, LSH], F32, isOutput=False)
    wv_t = dp("wv_t", [128, 2, 2, 128], F32, isOutput=False)     # K,hg,kc,M
    bv_p = dp("bv_p", [128, 2], F32, isOutput=False)
    woffx_t = dp("woffx_t", [128, 2, 128], F32, isOutput=False)  # K,kc,M
    woffy_t = dp("woffy_t", [128, 2, 128], F32, isOutput=False)
    boffx_p = dp("boffx_p", [128, 1], F32, isOutput=False)
    boffy_p = dp("boffy_p", [128, 1], F32, isOutput=False)
    wszx_t = dp("wszx_t", [128, 2, 8], F32, isOutput=False)
    wszy_t = dp("wszy_t", [128, 2, 8], F32, isOutput=False)
    bszx_p = dp("bszx_p", [8, 1], F32, isOutput=False)
    bszy_p = dp("bszy_p", [8, 1], F32, isOutput=False)
    watt_t = dp("watt_t", [128, 2, 128], F32, isOutput=False)    # K,kc,N
    batt_r = dp("batt_r", [1, 128], F32, isOutput=False)
    ones1 = dp("ones1", [1, 128], F32, isOutput=False)
    ident = dp("ident", [128, 128], F32, isOutput=False)
    wout_t = dp("wout_t", [128, 2, 2, 128], F32, isOutput=False)  # K,hg,oc,M
    wout4_t = dp("wout4_t", [128, 8, 2, 128], F32, isOutput=False)  # (dx,c),h,oc,o
    hbase = dp("hbase", [128, 1], F32, isOutput=False)
    bv_r = dp("bv_r", [1, 256], F32, isOutput=False)
    bn_s = dp("bn_s", [128, 2], F32, isOutput=False)
    bn_b = dp("bn_b", [128, 2], F32, isOutput=False)
    cenx2 = dp("cenx2", [128, LSH], F32, isOutput=False)
    ceny2 = dp("ceny2", [128, LSH], F32, isOutput=False)
    out_e = dp("out", [B, 2, 128, LSH], F32, isOutput=True)

    # DRAM scratch. idxd flat per (b,t): (h*16+p)*LSH + l
    #               ud flat per (b,t): ((h*16+p)*LSH + l)*DP + dx
    idxd_b = [nc.dram_tensor(f"idxd{b}", [NT, 128, LSH], I16)
              for b in range(B)]
    ud_b = [nc.dram_tensor(f"ud{b}", [NT, 128, LSH * DP], DT)
            for b in range(B)]
    szd = nc.dram_tensor("szd", [B, 2, 8, LSH], F32)
    MDPAD = 128                          # tail pad elems per (b,h) flat memory
    MDSZ = L * 32 + MDPAD                # memd per-(b,h) flat size (elems)
    CATSZ = NH * 4 * 1024 * 128          # memcat rows per b: 8h*4*1024, 128 e
    memd = nc.dram_tensor("memd", [B, NH, MDSZ], DT)
    memcat = nc.dram_tensor("memcat", [B, NH * 4 * 1024, 128], DT)
    ud2 = nc.dram_tensor("ud2", [B, NT, 2, 128, LSH], DT)

    AP = bass.AP
    Act = mybir.ActivationFunctionType
    Alu = mybir.AluOpType

    with tile.TileContext(nc) as tc, ExitStack() as ctx:
        consts = ctx.enter_context(tc.tile_pool(name="consts", bufs=1))
        featp = ctx.enter_context(tc.tile_pool(name="featp", bufs=4))
        memp = ctx.enter_context(tc.tile_pool(name="memp", bufs=1))
        prep = ctx.enter_context(tc.tile_pool(name="prep", bufs=1))
        gm = ctx.enter_context(tc.tile_pool(name="gm", bufs=1))
        gathp = ctx.enter_context(tc.tile_pool(name="gathp", bufs=3))
        ubcp = ctx.enter_context(tc.tile_pool(name="ubcp", bufs=2))
        idxwp = ctx.enter_context(tc.tile_pool(name="idxwp", bufs=2))
        outp = ctx.enter_context(tc.tile_pool(name="outp", bufs=2))
        ps_v = ctx.enter_context(tc.tile_pool(name="ps_v", bufs=2, space="PSUM"))
        ps_p = ctx.enter_context(tc.tile_pool(name="ps_p", bufs=1, space="PSUM"))
        ps_o = ctx.enter_context(tc.tile_pool(name="ps_o", bufs=1, space="PSUM"))

        def dmas(out, in_):          # bulk loads / ubc: SP queue
            nc.sync.dma_start(out=out, in_=in_)

        def dmaa(out, in_):          # small relayout traffic: ACT queue
            nc.scalar.dma_start(out=out, in_=in_)

        # ---------------- constants ----------------
        def cload(param, shape, dt=F32):
            t = consts.tile(list(shape), dt, tag=param.name, name=f"c_{param.name}")
            dmas(t[:], param.ap())
            return t
        wv_sb = cload(wv_t, [128, 2, 2, 128])
        bv_sb = cload(bv_p, [128, 2])
        woffx_sb = cload(woffx_t, [128, 2, 128])
        woffy_sb = cload(woffy_t, [128, 2, 128])
        boffx_sb = cload(boffx_p, [128, 1])
        boffy_sb = cload(boffy_p, [128, 1])
        wszx_sb = cload(wszx_t, [128, 2, 8])
        wszy_sb = cload(wszy_t, [128, 2, 8])
        bszx_sb = cload(bszx_p, [8, 1])
        bszy_sb = cload(bszy_p, [8, 1])
        watt_sb = cload(watt_t, [128, 2, 128])
        batt_sb = cload(batt_r, [1, 128])
        ones_sb = cload(ones1, [1, 128])
        id_sb = cload(ident, [128, 128])
        wout_f32 = cload(wout_t, [128, 2, 2, 128])
        bns_sb = cload(bn_s, [128, 2])
        bnb_sb = cload(bn_b, [128, 2])
        cenx_sb = cload(cenx2, [128, LSH])
        ceny_sb = cload(ceny2, [128, LSH])
        wout_sb = consts.tile([128, 2, 2, 128], DT)
        nc.vector.tensor_copy(out=wout_sb[:], in_=wout_f32[:])
        if dma_g:
            wout4_f32 = cload(wout4_t, [128, 8, 2, 128])
            wout4_sb = consts.tile([128, 8, 2, 128], DT)
            nc.vector.tensor_copy(out=wout4_sb[:], in_=wout4_f32[:])
            hbase_sb = cload(hbase, [128, 1])
            bvr_sb = cload(bv_r, [1, 256])
            zpad_sb = consts.tile([1, MDPAD], DT)
            nc.vector.memset(zpad_sb[:], 0.0)

        fsh_sb = consts.tile([128, B, 2, LSH], F32)
        dmas(fsh_sb[:], feat_sh.ap())

        # mem pairs/plain per (b, hg): [128, L, DP] (separate tiles so each
        # (b,hg)'s gathers depend only on its own value-conv writes).
        # d4: [128, L, 4] 2x2-patch stack, bufs=2 rotating slots.
        if dma_g:
            pass
        elif d4:
            mem_t = [[memp.tile([128, L, 4], DT, tag="quad",
                                name=f"quad{b}{hg}", bufs=2)
                      for hg in range(2)] for b in range(B)]
            for b in range(B):
                for hg in range(2):
                    nc.vector.memset(mem_t[b][hg][:, L - 65:, :], 0.0)
        else:
            mem_t = [[memp.tile([128, L, DP], DT, tag=f"mem{b}{hg}",
                                name=f"mem{b}{hg}") for hg in range(2)]
                     for b in range(B)]
            if d_pairs:
                for b in range(B):
                    for hg in range(2):
                        nc.vector.memset(mem_t[b][hg][:, L - 1:L, 1], 0.0)

        # ---------------- value conv ----------------
        if dma_g:
            # L-major: mem[b][128 l-part, 32 lc, 8 h, 32 c] bf16 -> DRAM memd
            # flat per (b,h) [l*32+c], then 4 shifted DRAM->DRAM copies into
            # memcat rows (h*4+s)*1024+j each holding 4 consecutive l x 32c.
            for b in range(B):
                memL = memp.tile([128, 32, 256], DT, tag="memL",
                                 name=f"memL{b}")
                for lc in range(32):
                    ps = ps_v.tile([128, 256], F32, tag="pv", name=f"pv{b}_{lc}")
                    fts = []
                    for kc in range(2):
                        ft = featp.tile([128, 128], F32, tag="ft")
                        dmas(ft[:], feat.ap()[:, b, kc, lc * 128:(lc + 1) * 128])
                        fts.append(ft)
                    for hgc in range(2):
                        sl = slice(hgc * 128, (hgc + 1) * 128)
                        for kc in range(2):
                            nc.tensor.matmul(ps[:, sl], fts[kc][:],
                                             wv_sb[:, hgc, kc, :],
                                             start=(kc == 0), stop=False)
                        nc.tensor.matmul(ps[:, sl], ones_sb[:],
                                         bvr_sb[:, sl], start=False, stop=True)
                    nc.scalar.activation(out=memL[:, lc, :], in_=ps[:],
                                         func=Act.Copy)
                for h in range(NH):
                    dmas(AP(tensor=memd, offset=(b * NH + h) * MDSZ,
                            ap=[[32, 128], [4096, 32], [1, 32]]),
                         memL[:, :, h * 32:(h + 1) * 32])
                    dmaa(AP(tensor=memd,
                            offset=(b * NH + h) * MDSZ + L * 32,
                            ap=[[1, 1], [1, MDPAD]]), zpad_sb[:])
                for s in range(4):
                    dmas(AP(tensor=memcat,
                            offset=b * CATSZ + s * 1024 * 128,
                            ap=[[1, 1], [4 * 1024 * 128, NH], [1, 131072]]),
                         AP(tensor=memd, offset=b * NH * MDSZ + 32 * s,
                            ap=[[1, 1], [MDSZ, NH], [1, 131072]]))
        else:
          for b in range(B):
            for hg in range(2):
                for n in range(L // 512):
                    ps = ps_v.tile([128, 512], F32, tag="pv")
                    for kc in range(2):
                        ft = featp.tile([128, 512], F32, tag="ft")
                        dmas(ft[:], feat.ap()[:, b, kc, n * 512:(n + 1) * 512])
                        nc.tensor.matmul(ps[:], wv_sb[:, hg, kc, :], ft[:],
                                         start=(kc == 0), stop=(kc == 1))
                    sl = slice(n * 512, (n + 1) * 512)
                    nc.scalar.activation(out=mem_t[b][hg][:, sl, 0], in_=ps[:],
                                         func=Act.Identity,
                                         bias=bv_sb[:, hg:hg + 1], scale=1.0)
                    if d4:
                        for dxi, sh in ((1, 1), (2, 64), (3, 65)):
                            if n == 0:
                                nc.scalar.activation(
                                    out=mem_t[b][hg][:, 0:512 - sh, dxi],
                                    in_=ps[:, sh:512], func=Act.Identity,
                                    bias=bv_sb[:, hg:hg + 1], scale=1.0)
                            else:
                                nc.scalar.activation(
                                    out=mem_t[b][hg][:, n * 512 - sh:
                                                     (n + 1) * 512 - sh, dxi],
                                    in_=ps[:], func=Act.Identity,
                                    bias=bv_sb[:, hg:hg + 1], scale=1.0)
                    elif d_pairs:
                        if n == 0:
                            nc.scalar.activation(
                                out=mem_t[b][hg][:, 0:511, 1], in_=ps[:, 1:512],
                                func=Act.Identity, bias=bv_sb[:, hg:hg + 1],
                                scale=1.0)
                        else:
                            nc.scalar.activation(
                                out=mem_t[b][hg][:, n * 512 - 1:(n + 1) * 512 - 1, 1],
                                in_=ps[:], func=Act.Identity,
                                bias=bv_sb[:, hg:hg + 1], scale=1.0)

        # ---------------- prep convs + grid math ----------------
        for b in range(B):
            offx = prep.tile([128, LSH], F32, tag="offx")
            offy = prep.tile([128, LSH], F32, tag="offy")
            for dst, wsb, bsb in ((offx, woffx_sb, boffx_sb),
                                  (offy, woffy_sb, boffy_sb)):
                ps = ps_p.tile([128, 512], F32, tag="pp")
                for kc in range(2):
                    nc.tensor.matmul(ps[:], wsb[:, kc, :], fsh_sb[:, b, kc, :],
                                     start=(kc == 0), stop=(kc == 1))
                nc.scalar.activation(out=dst[:], in_=ps[:], func=Act.Sigmoid,
                                     bias=bsb[:], scale=1.0)
            szx_b = prep.tile([128, LSH], F32, tag="szxb")
            szy_b = prep.tile([128, LSH], F32, tag="szyb")
            for k, (wsb, bsb, dstb) in enumerate(
                    ((wszx_sb, bszx_sb, szx_b), (wszy_sb, bszy_sb, szy_b))):
                ps = ps_p.tile([8, 512], F32, tag="pp", name="psz")
                for kc in range(2):
                    nc.tensor.matmul(ps[:], wsb[:, kc, :], fsh_sb[:, b, kc, :],
                                     start=(kc == 0), stop=(kc == 1))
                szs = gm.tile([8, LSH], F32, tag="szs")
                nc.scalar.activation(out=szs[:], in_=ps[:], func=Act.Sigmoid,
                                     bias=bsb[:], scale=1.0)
                nc.vector.tensor_scalar(out=szs[:], in0=szs[:], scalar1=0.75,
                                        scalar2=0.25, op0=Alu.min, op1=Alu.max)
                dmaa(AP(tensor=szd, offset=(b * 2 + k) * 8 * LSH,
                        ap=[[LSH, 8], [1, LSH]]), szs[:])
                for h in range(NH):
                    dmaa(dstb[h * 16:(h + 1) * 16, :],
                         AP(tensor=szd, offset=(b * 2 + k) * 8 * LSH + h * LSH,
                            ap=[[0, 16], [1, LSH]]))
            # attn conv (pixel-major) + softmax + transpose to (h,p)-major
            aT = prep.tile([128, LSH], F32, tag="aT")
            for lb in range(NLB):
                ps = ps_p.tile([128, 128], F32, tag="pp", name="pa")
                for kc in range(2):
                    nc.tensor.matmul(ps[:], fsh_sb[:, b, kc, lb * 128:(lb + 1) * 128],
                                     watt_sb[:, kc, :], start=(kc == 0), stop=False)
                nc.tensor.matmul(ps[:], ones_sb[:], batt_sb[:],
                                 start=False, stop=True)
                ae = gm.tile([128, 8, 16], F32, tag="ae")
                nc.scalar.activation(out=ae[:], in_=ps[:], func=Act.Exp)
                ssum = gm.tile([128, 8, 1], F32, tag="ssum")
                nc.vector.tensor_reduce(out=ssum[:], in_=ae[:],
                                        axis=mybir.AxisListType.X, op=Alu.add)
                nc.vector.reciprocal(out=ssum[:], in_=ssum[:])
                for h in range(NH):
                    nc.vector.tensor_scalar(out=ae[:, h, :], in0=ae[:, h, :],
                                            scalar1=ssum[:, h, :], scalar2=None,
                                            op0=Alu.mult)
                pst = ps_p.tile([128, 128], F32, tag="pp", name="pt")
                nc.tensor.transpose(pst[:], ae[:].rearrange("p a b -> p (a b)"),
                                    id_sb[:])
                nc.scalar.activation(out=aT[:, lb * 128:(lb + 1) * 128],
                                     in_=pst[:], func=Act.Copy)

            # ---- grid math, all tiles [128 (h,p), LSH] ----
            ixy = []
            for k, (off_k, szb, cen) in enumerate(((offx, szx_b, cenx_sb),
                                                   (offy, szy_b, ceny_sb))):
                t1 = gm.tile([128, LSH], F32, tag="t1")
                nc.vector.tensor_scalar(out=t1[:], in0=szb[:], scalar1=-0.5,
                                        scalar2=None, op0=Alu.mult)
                nc.vector.tensor_tensor(out=t1[:], in0=t1[:], in1=cen[:], op=Alu.add)
                g = gm.tile([128, LSH], F32, tag=f"g{k}")
                nc.vector.tensor_tensor(out=g[:], in0=off_k[:], in1=szb[:],
                                        op=Alu.mult)
                nc.vector.tensor_tensor(out=g[:], in0=g[:], in1=t1[:], op=Alu.add)
                nc.vector.tensor_scalar(out=g[:], in0=g[:], scalar1=1.0,
                                        scalar2=0.0, op0=Alu.min, op1=Alu.max)
                nc.vector.tensor_scalar(out=g[:], in0=g[:], scalar1=float(W - 1),
                                        scalar2=None, op0=Alu.mult)
                ixy.append(g)
            x0f, wxy = [], []
            for k in range(2):
                ci = gm.tile([128, LSH], I16, tag="ci")
                nc.vector.tensor_copy(out=ci[:], in_=ixy[k][:])
                cf = gm.tile([128, LSH], F32, tag=f"cf{k}")
                nc.vector.tensor_copy(out=cf[:], in_=ci[:])
                msk = gm.tile([128, LSH], F32, tag="msk")
                nc.vector.tensor_tensor(out=msk[:], in0=cf[:], in1=ixy[k][:],
                                        op=Alu.is_gt)
                nc.vector.tensor_tensor(out=cf[:], in0=cf[:], in1=msk[:],
                                        op=Alu.subtract)
                w = gm.tile([128, LSH], F32, tag=f"w{k}")
                nc.vector.tensor_tensor(out=w[:], in0=ixy[k][:], in1=cf[:],
                                        op=Alu.subtract)
                x0f.append(cf)
                wxy.append(w)
            y1f = gm.tile([128, LSH], F32, tag="y1f")
            nc.vector.tensor_scalar(out=y1f[:], in0=x0f[1][:], scalar1=1.0,
                                    scalar2=float(H - 1), op0=Alu.add, op1=Alu.min)
            flats = []
            for yf in (x0f[1], y1f):
                f = gm.tile([128, LSH], F32, tag=f"f{len(flats)}")
                nc.vector.tensor_scalar(out=f[:], in0=yf[:], scalar1=float(W),
                                        scalar2=None, op0=Alu.mult)
                nc.vector.tensor_tensor(out=f[:], in0=f[:], in1=x0f[0][:],
                                        op=Alu.add)
                flats.append(f)
            if not d_pairs:
                x1f = gm.tile([128, LSH], F32, tag="x1f")
                nc.vector.tensor_scalar(out=x1f[:], in0=x0f[0][:], scalar1=1.0,
                                        scalar2=float(W - 1), op0=Alu.add,
                                        op1=Alu.min)
                dxt = gm.tile([128, LSH], F32, tag="dxt")
                nc.vector.tensor_tensor(out=dxt[:], in0=x1f[:], in1=x0f[0][:],
                                        op=Alu.subtract)
                f01 = gm.tile([128, LSH], F32, tag="f01")
                nc.vector.tensor_tensor(out=f01[:], in0=flats[0][:], in1=dxt[:],
                                        op=Alu.add)
                f11 = gm.tile([128, LSH], F32, tag="f11")
                nc.vector.tensor_tensor(out=f11[:], in0=flats[1][:], in1=dxt[:],
                                        op=Alu.add)
                flats = [flats[0], f01, flats[1], f11]
            omx = gm.tile([128, LSH], F32, tag="omx")
            nc.vector.tensor_scalar(out=omx[:], in0=wxy[0][:], scalar1=-1.0,
                                    scalar2=1.0, op0=Alu.mult, op1=Alu.add)
            omy = gm.tile([128, LSH], F32, tag="omy")
            nc.vector.tensor_scalar(out=omy[:], in0=wxy[1][:], scalar1=-1.0,
                                    scalar2=1.0, op0=Alu.mult, op1=Alu.add)
            ay0 = gm.tile([128, LSH], F32, tag="ay0")
            nc.vector.tensor_tensor(out=ay0[:], in0=aT[:], in1=omy[:], op=Alu.mult)
            ay1 = gm.tile([128, LSH], F32, tag="ay1")
            nc.vector.tensor_tensor(out=ay1[:], in0=aT[:], in1=wxy[1][:],
                                    op=Alu.mult)
            if d4:
                uquad = gm.tile([128, LSH, 4], DT, tag="upair")
                for dxi, (yf, xf) in enumerate(((ay0, omx), (ay0, wxy[0]),
                                                (ay1, omx), (ay1, wxy[0]))):
                    uf = gm.tile([128, LSH], F32, tag="uf0")
                    nc.vector.tensor_tensor(out=uf[:], in0=yf[:], in1=xf[:],
                                            op=Alu.mult)
                    nc.vector.tensor_copy(out=uquad[:, :, dxi], in_=uf[:])
                dmaa(AP(tensor=ud_b[b], offset=0,
                        ap=[[LSH * 4, 128], [1, LSH * 4]]),
                     uquad[:].rearrange("p a b -> p (a b)"))
                fi = gm.tile([128, LSH], I16, tag="fi")
                nc.vector.tensor_copy(out=fi[:], in_=flats[0][:])
                dmaa(AP(tensor=idxd_b[b], offset=0,
                        ap=[[LSH, 128], [1, LSH]]), fi[:])
            for t, yf in (() if d4 else tuple(enumerate((ay0, ay1)))):
                uf0 = gm.tile([128, LSH], F32, tag="uf0")
                nc.vector.tensor_tensor(out=uf0[:], in0=yf[:], in1=omx[:],
                                        op=Alu.mult)
                uf1 = gm.tile([128, LSH], F32, tag="uf1")
                nc.vector.tensor_tensor(out=uf1[:], in0=yf[:], in1=wxy[0][:],
                                        op=Alu.mult)
                if dma_g:
                    # catrow = hbase + (flat%4)*1024 + flat//4
                    qt = gm.tile([128, LSH], F32, tag="qt")
                    nc.vector.tensor_scalar(out=qt[:], in0=flats[t][:],
                                            scalar1=0.25, scalar2=None,
                                            op0=Alu.mult)
                    qi = gm.tile([128, LSH], I16, tag="qi")
                    nc.vector.tensor_copy(out=qi[:], in_=qt[:])
                    qf = gm.tile([128, LSH], F32, tag="qf")
                    nc.vector.tensor_copy(out=qf[:], in_=qi[:])
                    mq = gm.tile([128, LSH], F32, tag="mq")
                    nc.vector.tensor_tensor(out=mq[:], in0=qf[:], in1=qt[:],
                                            op=Alu.is_gt)
                    nc.vector.tensor_tensor(out=qf[:], in0=qf[:], in1=mq[:],
                                            op=Alu.subtract)
                    cat = gm.tile([128, LSH], F32, tag="cat")
                    nc.vector.tensor_scalar(out=cat[:], in0=qf[:],
                                            scalar1=-4.0, scalar2=None,
                                            op0=Alu.mult)
                    nc.vector.tensor_tensor(out=cat[:], in0=cat[:],
                                            in1=flats[t][:], op=Alu.add)
                    nc.vector.tensor_scalar(out=cat[:], in0=cat[:],
                                            scalar1=1024.0,
                                            scalar2=hbase_sb[:],
                                            op0=Alu.mult, op1=Alu.add)
                    nc.vector.tensor_tensor(out=cat[:], in0=cat[:], in1=qf[:],
                                            op=Alu.add)
                    fi = gm.tile([128, LSH], I16, tag="fi")
                    nc.vector.tensor_copy(out=fi[:], in_=cat[:])
                    dmaa(AP(tensor=idxd_b[b], offset=t * 128 * LSH,
                            ap=[[LSH, 128], [1, LSH]]), fi[:])
                    for dxi, uu in enumerate((uf0, uf1)):
                        uc = gm.tile([128, LSH], DT, tag="uc")
                        nc.vector.tensor_copy(out=uc[:], in_=uu[:])
                        dmaa(AP(tensor=ud2,
                                offset=((b * NT + t) * 2 + dxi) * 128 * LSH,
                                ap=[[LSH, 128], [1, LSH]]), uc[:])
                elif d_pairs:
                    upair = gm.tile([128, LSH, 2], DT, tag="upair")
                    nc.vector.tensor_copy(out=upair[:, :, 0], in_=uf0[:])
                    nc.vector.tensor_copy(out=upair[:, :, 1], in_=uf1[:])
                    dmaa(AP(tensor=ud_b[b], offset=t * 128 * LSH * 2,
                            ap=[[LSH * 2, 128], [1, LSH * 2]]),
                         upair[:].rearrange("p a b -> p (a b)"))
                    fi = gm.tile([128, LSH], I16, tag="fi")
                    nc.vector.tensor_copy(out=fi[:], in_=flats[t][:])
                    dmaa(AP(tensor=idxd_b[b], offset=t * 128 * LSH,
                            ap=[[LSH, 128], [1, LSH]]), fi[:])
                else:
                    for dxi, uu in enumerate((uf0, uf1)):
                        tt = t * 2 + dxi
                        dmaa(AP(tensor=ud_b[b], offset=tt * 128 * LSH,
                                ap=[[LSH, 128], [1, LSH]]), uu[:])
                        fi = gm.tile([128, LSH], I16, tag="fi")
                        nc.vector.tensor_copy(out=fi[:], in_=flats[tt][:])
                        dmaa(AP(tensor=idxd_b[b], offset=tt * 128 * LSH,
                                ap=[[LSH, 128], [1, LSH]]), fi[:])

        # ---------------- gather + combine + out_proj ----------------
        if dma_g:
            # per (b, t, h): dma_gather of 8192 quad-runs -> [128=(dx,c), 8192]
            # stream (p, l); mult rows 0..63 by u (dx0/dx1 bcast over 32c);
            # 16 l-slot matmuls x 2 oc accumulate into psum[o, l].
            for gi in range(3):
                gz = gathp.tile([128, 1, NP * LSH], DT, tag="g4",
                                name=f"gz{gi}")
                nc.vector.memset(gz[64:128, :, :], 0.0)
            n_acc = NT * NH * NP      # matmuls per (b, oc) psum tile
            for b in range(B):
                pso = [ps_o.tile([128, 512], F32, tag=f"po{oc}",
                                 name=f"po{oc}_{b}") for oc in range(2)]
                cnt = [0, 0]
                for t in range(NT):
                    for h in range(NH):
                        idxw = idxwp.tile([128, 512], I16)
                        nc.vector.memset(idxw[:], 0)
                        for dup in range(2):
                            dmaa(idxw[dup * 16:dup * 16 + 16, :],
                                 AP(tensor=idxd_b[b],
                                    offset=t * 128 * LSH + h * 16 * LSH,
                                    ap=[[1, 16], [LSH, 16], [16, LSH // 16]]))
                        ubc = ubcp.tile([64, NP * LSH], DT)
                        for dxi in range(2):
                            dmas(ubc[dxi * 32:(dxi + 1) * 32, :],
                                 AP(tensor=ud2,
                                    offset=((b * NT + t) * 2 + dxi) * 128 * LSH
                                    + h * 16 * LSH,
                                    ap=[[0, 32], [1, NP * LSH]]))
                        g = gathp.tile([128, 1, NP * LSH], DT, tag="g4")
                        if stub_gather:
                            nc.gpsimd.dma_gather(
                                g[:, :, 0:128], AP(tensor=memcat,
                                                   offset=b * CATSZ,
                                                   ap=[[128, NH * 4 * 1024],
                                                       [1, 128]]),
                                idxw[:, 0:8], num_idxs=128,
                                num_idxs_reg=128, elem_size=128,
                                transpose=True)
                        else:
                            nc.gpsimd.dma_gather(
                                g[:], AP(tensor=memcat, offset=b * CATSZ,
                                         ap=[[128, NH * 4 * 1024], [1, 128]]),
                                idxw[:], num_idxs=NP * LSH,
                                num_idxs_reg=NP * LSH, elem_size=128,
                                transpose=True, single_packet=False)
                        nc.vector.tensor_tensor(out=g[0:64, :, :],
                                                in0=g[0:64, :, :],
                                                in1=ubc[:], op=Alu.mult)
                        for sl in range(NP):
                            for oc in range(2):
                                nc.tensor.matmul(
                                    pso[oc][:], wout4_sb[:, h, oc, :],
                                    g[:, 0, sl * 512:(sl + 1) * 512],
                                    start=(cnt[oc] == 0),
                                    stop=(cnt[oc] == n_acc - 1))
                                cnt[oc] += 1
                for oc in range(2):
                    o_sb = outp.tile([128, 512], F32, tag="osb")
                    nc.scalar.activation(out=o_sb[:], in_=pso[oc][:],
                                         func=Act.Identity,
                                         bias=bnb_sb[:, oc:oc + 1],
                                         scale=bns_sb[:, oc:oc + 1])
                    dmaa(AP(tensor=out_e,
                            offset=((b * 2 + oc) * 128) * LSH,
                            ap=[[LSH, 128], [1, LSH]]), o_sb[:])
        if d4:
            # stream per (b,h): i = (p, l_hi, q); one idx per point (2x2 quad);
            # dx handled by 4 stride-4-rhs matmuls into one [o, l] psum.
            n_acc4 = 2 * 8 * 2 * 4    # hg * chunks * slots * dx per (b, oc)
            for b in range(B):
                pso = [ps_o.tile([128, 512], F32, tag=f"po{oc}",
                                 name=f"po4_{oc}_{b}") for oc in range(2)]
                cnt = [0, 0]
                for hg in range(2):
                    idxw = idxwp.tile([128, 512], I16)
                    for hh in range(4):
                        h = hg * 4 + hh
                        for dup in range(2):
                            r = hh * 32 + dup * 16
                            dmaa(idxw[r:r + 16, :],
                                 AP(tensor=idxd_b[b], offset=h * 16 * LSH,
                                    ap=[[1, 16], [LSH, 16], [16, LSH // 16]]))
                    for pq in range(4):
                        ubc = ubcp.tile([128, 8192], DT)
                        for hh in range(4):
                            h = hg * 4 + hh
                            dmas(ubc[hh * 32:(hh + 1) * 32, :],
                                 AP(tensor=ud_b[b],
                                    offset=(h * 16 + pq * 4) * LSH * 4,
                                    ap=[[0, 32], [1, 8192]]))
                        for ci in range(2):
                            g = gathp.tile([128, 4096], DT)
                            c_glob = pq * 2 + ci
                            nc.gpsimd.ap_gather(
                                g[:, 0:64] if stub_gather else g[:],
                                mem_t[b][hg][:, :, :].rearrange(
                                    "p a b -> p (a b)"),
                                idxw[:, c_glob * 64:
                                     (c_glob * 64 + 1) if stub_gather
                                     else ((c_glob + 1) * 64)],
                                channels=128, num_elems=L, d=4,
                                num_idxs=(16 if stub_gather else CHUNK_I))
                            nc.vector.tensor_tensor(
                                out=g[:], in0=g[:],
                                in1=ubc[:, ci * 4096:(ci + 1) * 4096],
                                op=Alu.mult)
                            gap = g[:]
                            for sl in range(2):
                                for oc in range(2):
                                    for dx in range(4):
                                        rhs = AP(tensor=gap.tensor,
                                                 offset=gap.offset
                                                 + sl * 2048 + dx,
                                                 ap=[gap.ap[0], [4, 512]])
                                        nc.tensor.matmul(
                                            pso[oc][:], wout_sb[:, hg, oc, :],
                                            rhs, start=(cnt[oc] == 0),
                                            stop=(cnt[oc] == n_acc4 - 1))
                                        cnt[oc] += 1
                for oc in range(2):
                    o_sb = outp.tile([128, 512], F32, tag="osb", name=f"o4{b}{oc}")
                    nc.scalar.activation(out=o_sb[:], in_=pso[oc][:],
                                         func=Act.Identity,
                                         bias=bnb_sb[:, oc:oc + 1],
                                         scale=bns_sb[:, oc:oc + 1])
                    dmaa(AP(tensor=out_e, offset=((b * 2 + oc) * 128) * LSH,
                            ap=[[LSH, 128], [1, LSH]]), o_sb[:])

        # stream per (b,h): i = (t, p, l_hi, q); wrapped idx tile [16q, ...]
        n_acc = 2 * NT * 2 * 4 * 2    # matmuls per (b, oc, hf) psum tile
        for b in (range(B) if not (dma_g or d4) else []):
            pso = [[ps_o.tile([128, 256, 2] if d_pairs else [128, 512], F32,
                              tag=f"po{oc}{hf}", name=f"po{oc}{hf}{b}")
                    for hf in range(DP)] for oc in range(2)]
            cnt = [[0] * DP for _ in range(2)]
            for hg in range(2):
                idxw = idxwp.tile([128, NIDX_H // 16], I16)
                npg = NP * LSH // 16    # wrapped slots per tap-group
                for hh in range(4):
                    h = hg * 4 + hh
                    for dup in range(2):
                        r = hh * 32 + dup * 16
                        for t in range(NT):
                            dmaa(idxw[r:r + 16, t * npg:(t + 1) * npg],
                                 AP(tensor=idxd_b[b],
                                    offset=t * 128 * LSH + h * 16 * LSH,
                                    ap=[[1, 16], [LSH, 16], [16, LSH // 16]]))
                for t in range(NT):
                    for po in range(2):
                        ubc = ubcp.tile([128, 8 * LSH * DP], DT)
                        for hh in range(4):
                            h = hg * 4 + hh
                            dmas(ubc[hh * 32:(hh + 1) * 32, :],
                                 AP(tensor=ud_b[b],
                                    offset=t * 128 * LSH * DP
                                    + (h * 16 + po * 8) * LSH * DP,
                                    ap=[[0, 32], [1, 8 * LSH * DP]]))
                        for ci in range(4):
                            g = gathp.tile([128, CHUNK_I * DP], DT)
                            c_glob = t * 8 + po * 4 + ci
                            if stub_gather:
                                nc.gpsimd.ap_gather(
                                    g[:, 0:16 * DP],
                                    mem_t[b][hg][:, :, :].rearrange(
                                        "p a b -> p (a b)"),
                                    idxw[:, c_glob * 64:c_glob * 64 + 1],
                                    channels=128, num_elems=L, d=DP,
                                    num_idxs=16)
                            else:
                                nc.gpsimd.ap_gather(
                                    g[:],
                                    mem_t[b][hg][:, :, :].rearrange(
                                        "p a b -> p (a b)"),
                                    idxw[:, c_glob * 64:(c_glob + 1) * 64],
                                    channels=128, num_elems=L, d=DP,
                                    num_idxs=CHUNK_I)
                            nc.vector.tensor_tensor(
                                out=g[:], in0=g[:],
                                in1=ubc[:, ci * CHUNK_I * DP:(ci + 1) * CHUNK_I * DP],
                                op=Alu.mult)
                            for sl in range(2):
                                for oc in range(2):
                                    for hf in range(DP):
                                        lo = sl * 512 * DP + hf * 512
                                        po_t = pso[oc][hf]
                                        out_ap = (po_t[:].rearrange("p a b -> p (a b)")
                                                  if d_pairs else po_t[:])
                                        nc.tensor.matmul(
                                            out_ap, wout_sb[:, hg, oc, :],
                                            g[:, lo:lo + 512],
                                            start=(cnt[oc][hf] == 0),
                                            stop=(cnt[oc][hf] == n_acc - 1))
                                        cnt[oc][hf] += 1
            for oc in range(2):
                for hf in range(DP):
                    if d_pairs:
                        t0 = outp.tile([128, 256], F32, tag="t0f")
                        nc.scalar.activation(out=t0[:], in_=pso[oc][hf][:, :, 0],
                                             func=Act.Identity,
                                             bias=bnb_sb[:, oc:oc + 1],
                                             scale=bns_sb[:, oc:oc + 1])
                        t1 = outp.tile([128, 256], F32, tag="t1f")
                        nc.scalar.activation(out=t1[:], in_=pso[oc][hf][:, :, 1],
                                             func=Act.Copy, bias=0.0,
                                             scale=bns_sb[:, oc:oc + 1])
                        o_sb = outp.tile([128, 256], F32, tag="osb")
                        nc.vector.tensor_tensor(out=o_sb[:], in0=t0[:],
                                                in1=t1[:], op=Alu.add)
                        dmaa(AP(tensor=out_e,
                                offset=((b * 2 + oc) * 128) * LSH + hf * 256,
                                ap=[[LSH, 128], [1, 256]]), o_sb[:])
                    else:
                        o_sb = outp.tile([128, 512], F32, tag="osb")
                        nc.scalar.activation(out=o_sb[:], in_=pso[oc][hf][:],
                                             func=Act.Identity,
                                             bias=bnb_sb[:, oc:oc + 1],
                                             scale=bns_sb[:, oc:oc + 1])
                        dmaa(AP(tensor=out_e,
                                offset=((b * 2 + oc) * 128) * LSH,
                                ap=[[LSH, 128], [1, LSH]]), o_sb[:])

    nc.compile()
    _GRAPH_CACHE[key] = nc
    return nc


def _wout4(WoutT):
    # [128=(dx4,c32), 8h, 2oc, 128o]; dx>=2 rows are zero
    w4 = np.zeros((128, NH, 2, 128), np.float32)
    for dx in range(2):
        for c in range(32):
            w4[dx * 32 + c] = WoutT.reshape(NH, 32, 2, 128)[:, c]
    return np.ascontiguousarray(w4)


def stage_inputs(inputs, core):
    """Build the per-core in_map (all arrays pre-laid-out for plain DMAs)."""
    feat = np.ascontiguousarray(
        np.asarray(inputs['feat_sd'], np.float32).reshape(B, C, L))
    lo = core * LSH
    WvT = np.asarray(inputs['value_proj_w'], np.float32).T.copy()
    WoffT = np.asarray(inputs['anchor_deform_w'], np.float32).T.copy()
    WattT = np.asarray(inputs['anchor_att_w'], np.float32).T.copy()
    WszT = np.asarray(inputs['size_deform_w'], np.float32).T.copy()
    WoutT = np.asarray(inputs['out_proj_w'], np.float32).T.copy()
    boff = np.asarray(inputs['anchor_deform_b'], np.float32)
    bsz = np.asarray(inputs['size_deform_b'], np.float32)
    cols = (np.arange(W) + 0.5) / (W + EPS)
    rows = (np.arange(H) + 0.5) / (H + EPS)
    cx = np.tile(cols, H)[lo:lo + LSH].astype(np.float32)
    cy = np.repeat(rows, W)[lo:lo + LSH].astype(np.float32)
    fr = feat.reshape(B, 2, 128, L)
    m = {
        'feat': np.ascontiguousarray(fr.transpose(2, 0, 1, 3)),
        'feat_sh': np.ascontiguousarray(
            fr[:, :, :, lo:lo + LSH].transpose(2, 0, 1, 3)),
        'wv_t': np.ascontiguousarray(
            WvT.reshape(2, 128, 2, 128).transpose(1, 2, 0, 3)),
        'bv_p': np.ascontiguousarray(
            np.asarray(inputs['value_proj_b'], np.float32).reshape(2, 128).T),
        'woffx_t': np.ascontiguousarray(
            WoffT[:, 0::2].reshape(2, 128, 128).transpose(1, 0, 2)),
        'woffy_t': np.ascontiguousarray(
            WoffT[:, 1::2].reshape(2, 128, 128).transpose(1, 0, 2)),
        'boffx_p': np.ascontiguousarray(boff[0::2].reshape(128, 1)),
        'boffy_p': np.ascontiguousarray(boff[1::2].reshape(128, 1)),
        'wszx_t': np.ascontiguousarray(
            WszT[:, 0::2].reshape(2, 128, 8).transpose(1, 0, 2)),
        'wszy_t': np.ascontiguousarray(
            WszT[:, 1::2].reshape(2, 128, 8).transpose(1, 0, 2)),
        'bszx_p': np.ascontiguousarray(bsz[0::2].reshape(8, 1)),
        'bszy_p': np.ascontiguousarray(bsz[1::2].reshape(8, 1)),
        'watt_t': np.ascontiguousarray(
            WattT.reshape(2, 128, 128).transpose(1, 0, 2)),
        'batt_r': np.asarray(inputs['anchor_att_b'], np.float32).reshape(1, 128),
        'ones1': np.ones((1, 128), np.float32),
        'ident': np.eye(128, dtype=np.float32),
        'wout_t': np.ascontiguousarray(
            WoutT.reshape(2, 128, 2, 128).transpose(1, 0, 2, 3)),
        'wout4_t': _wout4(WoutT),
        'hbase': np.repeat(np.arange(NH, dtype=np.float32) * 4096.0,
                           16).reshape(128, 1),
        'bv_r': np.asarray(inputs['value_proj_b'], np.float32).reshape(1, 256),
        'bn_s': np.ascontiguousarray(
            (np.asarray(inputs['bn_gamma'], np.float32)
             / np.sqrt(np.float32(1.0 + 1e-5))).reshape(2, 128).T),
        'bn_b': np.ascontiguousarray(
            np.asarray(inputs['bn_beta'], np.float32).reshape(2, 128).T),
        'cenx2': np.ascontiguousarray(np.broadcast_to(cx, (128, LSH))),
        'ceny2': np.ascontiguousarray(np.broadcast_to(cy, (128, LSH))),
    }
    return m


def kernel(**inputs):
    nc = build_graph(use_bf16=True, d_pairs=True, d4=True)
    in_maps = [stage_inputs(inputs, i) for i in range(NCORES)]
    res = run_bass_kernel_spmd(nc, in_maps, core_ids=list(range(NCORES)))
    shards = [res.results[i]['out'].reshape(B, C, LSH) for i in range(NCORES)]
    full = np.concatenate(shards, axis=2).reshape(B, C, H, W)
    return full.astype(np.float32)



# revision 38
# speedup vs baseline: 1.6263x; 1.6263x over previous
"""Trainium2 Bass kernel for nn_AnchorDeformAtt (deformable anchor attention).

Sharding: spatial L-shard across 8 cores -- core i handles pixels
l in [512i, 512(i+1)) for BOTH batches and ALL heads. Zero collectives;
the host concatenates per-core output shards.

Design (cost-model driven):
  - Memory: x-pairs (m[j], m[j+1]) bf16, padded by 64 rows so the y1 tap
    stream is just idx+64. 16 ap_gathers per core (num_idxs=4096, d=2):
    cost max(in_free 8320, out_free 8192) / 0.6 -> ~11.5us each, ~185us
    Pool total (the bottleneck floor).
  - Value conv in bf16 (feat staged bf16 host-side); bias folded out
    (softmax x bilinear weights sum to 1 => Wout @ bv is a constant
    output bias, merged with bn_beta host-side; BN scale folded into
    Wout).
  - Gather stream per head 16-partition group: i = (p, lhi, lq) with
    row q = lq = l%16, cols (p, lhi). y0 indices are wrapped at STORE
    time into DRAM [k][hg][b][h][lq][p][lhi] (strided store pays the
    transpose tax once); loads are clean 2-dim DMAs; y1 index tiles are
    y0 tiles + 64 computed on DVE.
  - u = attn * bilinear weights staged as [b][(h,p)][tap][l][dx],
    broadcast to each head's 32 channels with 3-dim-AP DMAs.
  - Combine matmuls (contraction over 4 heads x 32 ch) accumulate
    psum[o, 256l] over (hg, tap, p, dx); one ACT copy + DMA out per
    (b, lblk, oc).
  - Emission interleaves prep phases between gather blocks so Pool
    never starves; idx stores ride SP behind ft loads, idxw loads on
    ACT.
"""
from contextlib import ExitStack

import numpy as np
import ml_dtypes

import concourse.bass as bass
import concourse.mybir as mybir
import concourse.tile as tile
from concourse import bacc
from concourse.bass_utils import run_bass_kernel_spmd

NH, NP = 8, 16
B, C, H, W = 2, 256, 64, 64
L = H * W            # 4096
NCORES = 8
LSH = L // NCORES    # 512
LPAD = L + 64        # pairs-memory rows (y1 tap reads idx+64)
EPS = 1e-6
F32 = mybir.dt.float32
BF16 = mybir.dt.bfloat16
I16 = mybir.dt.int16
F32R = mybir.dt.float32r

_GRAPH_CACHE = {}

PARAM_SPECS = {
    'featb': ([128, B, 2, L], BF16),
    'fsh': ([128, B, 2, LSH], F32R),
    'wv_t': ([128, 2, 2, 128], BF16),    # K, hg, kc, M
    'woff_t': ([128, 2, 2, 128], F32R),   # K, xy, kc, M
    'boff_p': ([128, 2], F32),           # per-partition bias, xy
    'wsz_t': ([128, 2, 2, 8], F32R),      # K, xy, kc, 8
    'bsz_p': ([8, 2], F32),
    'sel8': ([8, 128], F32),             # sel8[h, h*16+p] = 1
    'watt_t': ([128, 2, 128], F32R),      # K, kc, N
    'batt_r': ([1, 128], F32),
    'ones1': ([1, 128], F32),
    'ident': ([128, 128], F32),
    'wout_t': ([128, 2, 2, 128], BF16),  # K, hg, oc, M (bn-scaled)
    'obias': ([128, 2], F32),            # (Wout_sc @ bv + beta) as [m, oc]
    'cen2': ([128, 2, LSH], F32),        # packed (x|y) centers
}


def build_graph(stub_gather=False):
    key = (stub_gather,)
    if key in _GRAPH_CACHE:
        return _GRAPH_CACHE[key]

    nc = bacc.Bacc("TRN2", target_bir_lowering=False, debug=False,
                   num_devices=NCORES)
    dp = nc.declare_dram_parameter
    P = {n: dp(n, s, dt, isOutput=False) for n, (s, dt) in PARAM_SPECS.items()}
    out_e = dp("out", [B, 2, 128, LSH], F32, isOutput=True)

    # wrapped y0 idx staging, one tensor per (k, hg): [b][h4][lq][p][lhi]
    idxd = {(k, hg): nc.dram_tensor(f"idxd{k}{hg}", [B, 4, 16, 16, 16], I16)
            for k in range(2) for hg in range(2)}
    # u staging: [b][(h,p) 128][tap 2][l 512][dx 2]
    ud = nc.dram_tensor("ud", [B, 128, 2, LSH, 2], BF16)

    AP = bass.AP
    Act = mybir.ActivationFunctionType
    Alu = mybir.AluOpType

    with tile.TileContext(nc) as tc, ExitStack() as ctx:
        consts = ctx.enter_context(tc.tile_pool(name="consts", bufs=1))
        featp = ctx.enter_context(tc.tile_pool(name="featp", bufs=4))
        fshp = ctx.enter_context(tc.tile_pool(name="fshp", bufs=2))
        memp = ctx.enter_context(tc.tile_pool(name="memp", bufs=1))
        prep = ctx.enter_context(tc.tile_pool(name="prep", bufs=1))
        gm = ctx.enter_context(tc.tile_pool(name="gm", bufs=1))
        idxwp = ctx.enter_context(tc.tile_pool(name="idxwp", bufs=2))
        ubcp = ctx.enter_context(tc.tile_pool(name="ubcp", bufs=2))
        gathp = ctx.enter_context(tc.tile_pool(name="gathp", bufs=3))
        outp = ctx.enter_context(tc.tile_pool(name="outp", bufs=2))
        ps_v = ctx.enter_context(tc.tile_pool(name="ps_v", bufs=2, space="PSUM"))
        ps_p = ctx.enter_context(tc.tile_pool(name="ps_p", bufs=2, space="PSUM"))
        ps_o = ctx.enter_context(tc.tile_pool(name="ps_o", bufs=1, space="PSUM"))

        def dmas(out, in_):          # SP queue: consts, ft, idx stores, ubc
            nc.sync.dma_start(out=out, in_=in_)

        def dmaa(out, in_):          # ACT queue: fsh, idxw loads, uq, out
            nc.scalar.dma_start(out=out, in_=in_)

        # ---------------- constants ----------------
        def cload(name):
            shape, dt = PARAM_SPECS[name]
            t = consts.tile(list(shape), dt, tag=name, name=f"c_{name}")
            dmas(t[:], P[name].ap())
            return t

        wv_sb = cload('wv_t')
        woff_sb = cload('woff_t')
        boff_sb = cload('boff_p')
        wsz_sb = cload('wsz_t')
        bsz_sb = cload('bsz_p')
        sel8_sb = cload('sel8')
        watt_sb = cload('watt_t')
        batt_sb = cload('batt_r')
        ones_sb = cload('ones1')
        id_sb = cload('ident')
        wout_sb = cload('wout_t')
        obias_sb = cload('obias')
        cen_sb = cload('cen2')

        mem_t = {}
        st = {}

        # ---------------- value conv (bf16 pairs, no bias) ----------------
        def conv_sec(b, hg):
            q = memp.tile([128, LPAD, 2], BF16, tag="pair",
                          name=f"pair{b}{hg}", bufs=2)
            mem_t[(b, hg)] = q
            nc.vector.memset(q[:, L - 1:, :], 0.0)
            for c in range(4):
                ft = featp.tile([128, 2, 1024], BF16, tag="ft")
                dmas(ft[:], AP(tensor=P['featb'], offset=b * 2 * L + c * 1024,
                               ap=[[B * 2 * L, 128], [L, 2], [1, 1024]]))
                for j in range(2):
                    n = c * 2 + j
                    ps = ps_v.tile([128, 512], F32, tag="pv")
                    for kc in range(2):
                        nc.tensor.matmul(ps[:], wv_sb[:, hg, kc, :],
                                         ft[:, kc, j * 512:(j + 1) * 512],
                                         start=(kc == 0), stop=(kc == 1))
                    nc.scalar.activation(out=q[:, n * 512:(n + 1) * 512, 0],
                                         in_=ps[:], func=Act.Copy)
                    if n == 0:
                        nc.vector.tensor_copy(out=q[:, 0:511, 1],
                                              in_=ps[:, 1:512])
                    else:
                        nc.vector.tensor_copy(
                            out=q[:, n * 512 - 1:(n + 1) * 512 - 1, 1],
                            in_=ps[:])

        # ---------------- prep phases (per b) ----------------
        def prep_a(b):
            """fsh load; offset+size convs -> packed offp/szbp [128,2,LSH]."""
            s = st.setdefault(b, {})
            fsh = fshp.tile([128, 2, LSH], F32R, tag="fsh")
            dmaa(fsh[:], P['fsh'].ap()[:, b, :, :])
            s['fsh'] = fsh
            offp = prep.tile([128, 2, LSH], F32, tag="offp")
            szbp = prep.tile([128, 2, LSH], F32, tag="szbp")
            s['offp'], s['szbp'] = offp, szbp
            for xy in range(2):
                ps = ps_p.tile([128, 512], F32, tag="pp", name="psz")
                for kc in range(2):
                    nc.tensor.matmul(ps[0:8, :], wsz_sb[:, xy, kc, :],
                                     fsh[:, kc, :], start=(kc == 0),
                                     stop=(kc == 1))
                szs = gm.tile([8, LSH], F32, tag="szs")
                nc.scalar.activation(out=szs[:], in_=ps[0:8, :],
                                     func=Act.Sigmoid,
                                     bias=bsz_sb[:, xy:xy + 1], scale=1.0)
                nc.vector.tensor_scalar(out=szs[:], in0=szs[:], scalar1=0.75,
                                        scalar2=0.25, op0=Alu.min, op1=Alu.max)
                psb = ps_p.tile([128, 512], F32, tag="pp", name="psb")
                nc.tensor.matmul(psb[:], sel8_sb[:], szs[:],
                                 start=True, stop=True)
                nc.vector.tensor_copy(out=szbp[:, xy, :], in_=psb[:])
                ps2 = ps_p.tile([128, 512], F32, tag="pp", name="po")
                for kc in range(2):
                    nc.tensor.matmul(ps2[:], woff_sb[:, xy, kc, :],
                                     fsh[:, kc, :], start=(kc == 0),
                                     stop=(kc == 1))
                nc.scalar.activation(out=offp[:, xy, :], in_=ps2[:],
                                     func=Act.Sigmoid,
                                     bias=boff_sb[:, xy:xy + 1], scale=1.0)

        def prep_c(b):
            """grid -> floor -> flat y0 idx (packed x|y in one [128,2,LSH])."""
            s = st[b]
            offp, szbp = s['offp'], s['szbp']
            o2 = offp[:].rearrange("p a b -> p (a b)")
            s2 = szbp[:].rearrange("p a b -> p (a b)")
            cf = gm.tile([128, 2, LSH], F32, tag="cf")
            c2 = cf[:].rearrange("p a b -> p (a b)")
            ci = gm.tile([128, 2, LSH], I16, tag="ci")
            i2 = ci[:].rearrange("p a b -> p (a b)")
            msk = gm.tile([128, 2, LSH], F32, tag="msk")
            m2 = msk[:].rearrange("p a b -> p (a b)")
            nc.vector.tensor_scalar(out=o2, in0=o2, scalar1=-0.5,
                                    scalar2=None, op0=Alu.add)
            nc.vector.tensor_tensor(out=o2, in0=o2, in1=s2, op=Alu.mult)
            nc.vector.tensor_tensor(
                out=o2, in0=o2,
                in1=cen_sb[:].rearrange("p a b -> p (a b)"), op=Alu.add)
            nc.vector.tensor_scalar(out=o2, in0=o2, scalar1=1.0, scalar2=0.0,
                                    op0=Alu.min, op1=Alu.max)
            nc.vector.tensor_scalar(out=o2, in0=o2, scalar1=float(W - 1),
                                    scalar2=None, op0=Alu.mult)
            nc.vector.tensor_copy(out=i2, in_=o2)
            nc.vector.tensor_copy(out=c2, in_=i2)
            nc.vector.tensor_tensor(out=m2, in0=c2, in1=o2, op=Alu.is_gt)
            nc.vector.tensor_tensor(out=c2, in0=c2, in1=m2, op=Alu.subtract)
            nc.vector.tensor_tensor(out=o2, in0=o2, in1=c2, op=Alu.subtract)
            # flat y0 = y0f*W + x0f -> reuse szbp x-half as scratch, fi -> ci
            fl = szbp[:, 0, :]
            nc.vector.tensor_scalar(out=fl, in0=cf[:, 1, :],
                                    scalar1=float(W), scalar2=None,
                                    op0=Alu.mult)
            nc.vector.tensor_tensor(out=fl, in0=fl, in1=cf[:, 0, :],
                                    op=Alu.add)
            fi = gm.tile([128, LSH], I16, tag="fi")
            nc.vector.tensor_copy(out=fi[:], in_=fl)
            s['fi'] = fi
            # wx/wy in offp halves; cf/msk slots free for prep_d reuse
            s['cf'], s['msk'] = cf, msk

        def prep_cs(b, k):
            """Wrapped y0 idx stores for l-block k + clean idxw loads +
            DVE-derived y1 idx tiles."""
            s = st[b]
            fi = s['fi']
            for hg in range(2):
                for hh in range(4):
                    h = hg * 4 + hh
                    dmas(AP(tensor=idxd[(k, hg)], offset=(b * 4 + hh) * 4096,
                            ap=[[16, 16], [1, 16], [256, 16]]),
                         fi[h * 16:(h + 1) * 16, k * 256:(k + 1) * 256])
                ix = idxwp.tile([128, 256], I16, tag=f"ix{hg}{k}",
                                name=f"ix{b}{hg}{k}")
                st[('ix', b, hg, k)] = ix
                for hh in range(4):
                    for dup in range(2):
                        r = hh * 32 + dup * 16
                        dmaa(ix[r:r + 16, :],
                             AP(tensor=idxd[(k, hg)],
                                offset=(b * 4 + hh) * 4096,
                                ap=[[256, 16], [1, 256]]))


        def prep_b(b):
            """attn conv (pixel-major) + softmax + transpose -> aT."""
            s = st[b]
            fsh = s['fsh']
            aT = prep.tile([128, LSH], F32, tag="aT")
            s['aT'] = aT
            for lb in range(LSH // 128):
                ps = ps_p.tile([128, 128], F32, tag="pp", name="pa")
                for kc in range(2):
                    nc.tensor.matmul(ps[:], fsh[:, kc, lb * 128:(lb + 1) * 128],
                                     watt_sb[:, kc, :], start=(kc == 0),
                                     stop=False)
                nc.tensor.matmul(ps[:], ones_sb[:], batt_sb[:],
                                 start=False, stop=True)
                ae = gm.tile([128, 8, 16], F32, tag="ae")
                nc.scalar.activation(out=ae[:], in_=ps[:], func=Act.Exp)
                ssum = gm.tile([128, 8, 1], F32, tag="ssum")
                nc.vector.tensor_reduce(out=ssum[:], in_=ae[:],
                                        axis=mybir.AxisListType.X, op=Alu.add)
                nc.vector.reciprocal(out=ssum[:], in_=ssum[:])
                for h in range(NH):
                    nc.vector.tensor_scalar(out=ae[:, h, :], in0=ae[:, h, :],
                                            scalar1=ssum[:, h, :],
                                            scalar2=None, op0=Alu.mult)
                pst = ps_p.tile([128, 128], F32, tag="pp", name="pt")
                nc.tensor.transpose(pst[:], ae[:].rearrange("p a b -> p (a b)"),
                                    id_sb[:])
                nc.scalar.activation(out=aT[:, lb * 128:(lb + 1) * 128],
                                     in_=pst[:], func=Act.Copy)

        def prep_d(b):
            """u = attn * bilinear -> upair [128, tap, l, dx] -> DRAM."""
            s = st[b]
            offp, cf, msk, aT = s['offp'], s['cf'], s['msk'], s['aT']
            wx, wy = offp[:, 0, :], offp[:, 1, :]
            omx, omy = msk[:, 0, :], msk[:, 1, :]
            ay0, ay1 = cf[:, 0, :], cf[:, 1, :]
            nc.vector.tensor_scalar(out=omx, in0=wx, scalar1=-1.0,
                                    scalar2=1.0, op0=Alu.mult, op1=Alu.add)
            nc.vector.tensor_scalar(out=omy, in0=wy, scalar1=-1.0,
                                    scalar2=1.0, op0=Alu.mult, op1=Alu.add)
            nc.vector.tensor_tensor(out=ay0, in0=aT[:], in1=omy, op=Alu.mult)
            nc.vector.tensor_tensor(out=ay1, in0=aT[:], in1=wy, op=Alu.mult)
            uq = gm.tile([128, 2, LSH, 2], BF16, tag="uq")
            for tap, dxi, yf, xf in ((0, 0, ay0, omx), (0, 1, ay0, wx),
                                     (1, 0, ay1, omx), (1, 1, ay1, wx)):
                nc.vector.tensor_tensor(out=uq[:, tap, :, dxi], in0=yf,
                                        in1=xf, op=Alu.mult)
            dmaa(AP(tensor=ud, offset=b * 128 * 2048,
                    ap=[[2048, 128], [1, 2048]]),
                 uq[:].rearrange("p a b c -> p (a b c)"))

        # ---------------- gather + combine ----------------
        pso = {}
        cnt = {}

        def gblock(b, hg, tap, k):
            pair = mem_t[(b, hg)]
            idxw = st[('ix', b, hg, k)]
            # y1 tap: same indices, memory viewed at +64 rows
            src = (pair[:, 64:, :] if tap else pair[:, 0:L, :]).rearrange(
                "p a b -> p (a b)")
            g = gathp.tile([128, 4096, 2], BF16, tag="g2")
            if stub_gather:
                nc.gpsimd.ap_gather(
                    g[:, 0:16, :], src, idxw[:, 0:1], channels=128,
                    num_elems=L, d=2, num_idxs=16)
            else:
                nc.gpsimd.ap_gather(
                    g[:], src, idxw[:], channels=128,
                    num_elems=L, d=2, num_idxs=4096)
            ubc = ubcp.tile([128, 8192], BF16, tag="ubc")
            for hh in range(4):
                dmas(ubc[hh * 32:(hh + 1) * 32, :],
                     AP(tensor=ud,
                        offset=(b * 128 + (hg * 4 + hh) * 16) * 2048
                        + tap * 1024 + k * 512,
                        ap=[[0, 32], [2048, 16], [1, 512]]))
            nc.vector.tensor_tensor(
                out=g[:].rearrange("p a b -> p (a b)"),
                in0=g[:].rearrange("p a b -> p (a b)"),
                in1=ubc[:], op=Alu.mult)
            for oc in range(2):
                if (b, k, oc) not in pso:
                    pso[(b, k, oc)] = ps_o.tile([128, 256, 2], F32,
                                                tag=f"po{k}{oc}",
                                                name=f"po{b}{k}{oc}")
                    cnt[(b, k, oc)] = 0
            gap = g[:]
            for oc in range(2):
                for p in range(16):
                    rhs = AP(tensor=gap.tensor, offset=gap.offset + p * 512,
                             ap=[gap.ap[0], [1, 512]])
                    c = cnt[(b, k, oc)]
                    nc.tensor.matmul(
                        pso[(b, k, oc)][:].rearrange("p a b -> p (a b)"),
                        wout_sb[:, hg, oc, :], rhs,
                        start=(c == 0), stop=(c == 63))
                    cnt[(b, k, oc)] = c + 1

        def finalize(b, k):
            for oc in range(2):
                o_sb = outp.tile([128, 256, 1], F32, tag="osb")
                nc.vector.tensor_reduce(out=o_sb[:],
                                        in_=pso[(b, k, oc)][:],
                                        axis=mybir.AxisListType.X, op=Alu.add)
                o2 = o_sb[:].rearrange("p a b -> p (a b)")
                nc.vector.tensor_scalar(out=o2, in0=o2,
                                        scalar1=obias_sb[:, oc:oc + 1],
                                        scalar2=None, op0=Alu.add)
                dmaa(AP(tensor=out_e,
                        offset=((b * 2 + oc) * 128) * LSH + k * 256,
                        ap=[[LSH, 128], [1, 256]]), o_sb[:])

        # ---------------- emission schedule ----------------
        prep_a(0)
        prep_c(0)
        conv_sec(0, 0)
        conv_sec(0, 1)
        prep_cs(0, 0)
        prep_b(0)
        prep_d(0)
        gblock(0, 0, 0, 0)
        prep_cs(0, 1)
        gblock(0, 0, 1, 0)
        prep_a(1)
        gblock(0, 0, 0, 1)
        gblock(0, 0, 1, 1)
        prep_b(1)
        conv_sec(1, 0)
        gblock(0, 1, 0, 0)
        prep_c(1)
        prep_cs(1, 0)
        gblock(0, 1, 1, 0)
        finalize(0, 0)
        gblock(0, 1, 0, 1)
        prep_d(1)
        gblock(0, 1, 1, 1)
        finalize(0, 1)
        prep_cs(1, 1)
        conv_sec(1, 1)
        gblock(1, 0, 0, 0)
        gblock(1, 0, 1, 0)
        gblock(1, 0, 0, 1)
        gblock(1, 0, 1, 1)
        gblock(1, 1, 0, 0)
        gblock(1, 1, 1, 0)
        finalize(1, 0)
        gblock(1, 1, 0, 1)
        gblock(1, 1, 1, 1)
        finalize(1, 1)

    nc.compile()
    _GRAPH_CACHE[key] = nc
    return nc


def stage_inputs(inputs, core):
    """Build the per-core in_map (all arrays pre-laid-out for plain DMAs)."""
    bf16 = ml_dtypes.bfloat16
    feat = np.ascontiguousarray(
        np.asarray(inputs['feat_sd'], np.float32).reshape(B, C, L))
    lo = core * LSH
    WvT = np.asarray(inputs['value_proj_w'], np.float32).T.copy()
    WoffT = np.asarray(inputs['anchor_deform_w'], np.float32).T.copy()
    WattT = np.asarray(inputs['anchor_att_w'], np.float32).T.copy()
    WszT = np.asarray(inputs['size_deform_w'], np.float32).T.copy()
    WoutT = np.asarray(inputs['out_proj_w'], np.float32).T.copy()
    boff = np.asarray(inputs['anchor_deform_b'], np.float32)
    bsz = np.asarray(inputs['size_deform_b'], np.float32)
    bv = np.asarray(inputs['value_proj_b'], np.float32)
    bn_s = (np.asarray(inputs['bn_gamma'], np.float32)
            / np.sqrt(np.float32(1.0 + 1e-5)))
    beta = np.asarray(inputs['bn_beta'], np.float32)
    WoutT_sc = WoutT * bn_s[None, :]
    obias = (bv @ WoutT_sc + beta).reshape(2, 128).T
    sel8 = np.zeros((8, 128), np.float32)
    for h in range(8):
        sel8[h, h * 16:(h + 1) * 16] = 1.0
    cols = (np.arange(W) + 0.5) / (W + EPS)
    rows = (np.arange(H) + 0.5) / (H + EPS)
    cx = np.tile(cols, H)[lo:lo + LSH].astype(np.float32)
    cy = np.repeat(rows, W)[lo:lo + LSH].astype(np.float32)
    cen2 = np.stack([np.broadcast_to(cx, (128, LSH)),
                     np.broadcast_to(cy, (128, LSH))], axis=1)
    # woff/wsz packed: xy-interleaved output channels split into x|y planes
    woff = np.stack([WoffT[:, 0::2], WoffT[:, 1::2]],
                    axis=1)                      # [256, 2, 128]
    woff_t = woff.reshape(2, 128, 2, 128).transpose(1, 2, 0, 3)
    wsz = np.stack([WszT[:, 0::2], WszT[:, 1::2]], axis=1)  # [256, 2, 8]
    wsz_t = wsz.reshape(2, 128, 2, 8).transpose(1, 2, 0, 3)
    fr = feat.reshape(B, 2, 128, L)
    m = {
        'featb': np.ascontiguousarray(
            fr.transpose(2, 0, 1, 3)).astype(bf16),
        'fsh': np.ascontiguousarray(
            fr[:, :, :, lo:lo + LSH].transpose(2, 0, 1, 3)),
        'wv_t': np.ascontiguousarray(
            WvT.reshape(2, 128, 2, 128).transpose(1, 2, 0, 3)).astype(bf16),
        'woff_t': np.ascontiguousarray(woff_t),
        'boff_p': np.ascontiguousarray(
            np.stack([boff[0::2], boff[1::2]], axis=1)),
        'wsz_t': np.ascontiguousarray(wsz_t),
        'bsz_p': np.ascontiguousarray(
            np.stack([bsz[0::2], bsz[1::2]], axis=1)),
        'sel8': sel8,
        'watt_t': np.ascontiguousarray(
            WattT.reshape(2, 128, 128).transpose(1, 0, 2)),
        'batt_r': np.asarray(inputs['anchor_att_b'],
                             np.float32).reshape(1, 128),
        'ones1': np.ones((1, 128), np.float32),
        'ident': np.eye(128, dtype=np.float32),
        'wout_t': np.ascontiguousarray(
            WoutT_sc.reshape(2, 128, 2, 128).transpose(1, 0, 2, 3)
        ).astype(bf16),
        'obias': np.ascontiguousarray(obias),
        'cen2': np.ascontiguousarray(cen2),
    }
    return m


def kernel(**inputs):
    nc = build_graph()
    in_maps = [stage_inputs(inputs, i) for i in range(NCORES)]
    res = run_bass_kernel_spmd(nc, in_maps, core_ids=list(range(NCORES)))
    shards = [res.results[i]['out'].reshape(B, C, LSH) for i in range(NCORES)]
    full = np.concatenate(shards, axis=2).reshape(B, C, H, W)
    return full.astype(np.float32)
